# revision 16
# baseline (speedup 1.0000x reference)
"""Multi-head attention (B=2, N=4096, C=512, H=8, D=64) on 8 TRN2 NeuronCores.

Sharding: data-parallel over batch (2 groups of 4 cores) x tensor-parallel over
heads (2 heads/core). Per core: qkv projection, attention for its 2 heads, and
a partial output projection y_partial^T = Wp_slice^T @ attn^T; the host sums
the 4 per-batch partials, transposes, adds bias.

Engine layout learned from profiling:
- All matmuls bf16; x / w_qkv / w_proj are cast to bf16 on the HOST.
- DMA descriptor dispatch (~30-40 ns/descriptor, shared DGE) paces all loads
  and stores, so every DRAM tensor is packed so each SBUF partition's data is
  one long contiguous DRAM run: x is a single [128, 16384] transfer
  (128 x 32 KB descriptors), wqkv one [128, 1536], and the output is stored
  as [128, 2048] rows per n-block (8 KB descriptors).
- exp(S) runs on ScalarE out of 3-bank (128,1536) PSUM tiles at the
  documented (cols+352)/1.2 ns; two tiles per group are offloaded to VectorE
  via a custom 2-instruction exp: p = relu(cubic(s)) then p^16 (4 squarings),
  which matches exp(SCALE*s) to ~4e-4 over the live logit range. Their PV
  matmuls are deferred 3 tiles to cover the longer DVE latency.
- Softmax denominator rides the PV matmul as a ones-column appended to V
  (lhsT is [V_h | 1], M=65); the denominator row is moved to partition 0 by a
  small DMA, inverted on VectorE, and partition-broadcast on GpSimd.
"""
import os
import sys

for _p in ("/opt/trn_rl_repo", "/root/.axon_site/_ro/trn_rl_repo"):
    if os.path.isdir(_p) and _p not in sys.path:
        sys.path.append(_p)

import numpy as np
import ml_dtypes
from contextlib import ExitStack

import concourse.bass as bass
import concourse.mybir as mybir
import concourse.tile as tile
from concourse import bacc
from concourse.bass_utils import run_bass_kernel_spmd

F32 = mybir.dt.float32
BF16 = mybir.dt.bfloat16
EXP = mybir.ActivationFunctionType.Exp

DIM, N, HD = 512, 4096, 64
SCALE = HD ** -0.5
NB = N // 512    # 8  n-blocks of 512 queries
MB = N // 128    # 32 m-chunks of 128 keys
CC = DIM // 128  # 4  c-chunks of the model dim
# m-chunks per (nb, h) are processed in 3-bank PSUM tiles of 3 chunks each
TILES = [(t * 3, 3) for t in range(10)] + [(30, 2)]

# ---- custom DVE exp: es = (relu(cubic(s)))^16 ~= exp(SCALE*s) -------------
# cubic fit of 2^(k*s), k = SCALE*log2(e)/16, minimax-relative over logits
# s*SCALE in [-9.5, 7.2]; worst rel err ^16 ~ 6e-3, and it sits at large
# negative logits where softmax weights are ~0 (full-pipeline err is ~3e-3).
_EC3, _EC2, _EC1, _EC0 = (
    7.29580735e-08, 3.10468742e-05, 7.82950121e-03, 9.99782936e-01,
)


def _register_exp_ops():
    from concourse import dve_ops as dvo
    from concourse.dve_spec import Spec, Src0, relu, sq, C0, C1, C2, C3

    def reg(name, spec, subdim=False):
        for o in dvo.OPS:
            if o.name == name:
                return o
        opcode = max(dvo._SUB_OPCODE_FOR_NAME.values()) + 1
        from concourse.dve_uop import DveOpSpec

        shas = {}
        for ver in ("v3", "v4"):
            try:
                r = DveOpSpec(
                    name=name,
                    opcode=opcode,
                    uops=dvo.lower(spec, ver=ver),
                    rd1_en=dvo.has_src1(spec),
                )
                shas[ver] = r.sha(ver)
            except Exception:
                pass
        op = dvo.DveOp(name, spec, subdim=subdim, uops_sha=shas)
        dvo.OPS.append(op)
        dvo.CUSTOM_DVE_SPECS[name] = spec
        dvo._SUB_OPCODE_FOR_NAME[name] = opcode
        return op

    p1 = reg(
        "EXP16_P1_ANT",
        Spec(
            body=dvo._spill_c3_to_src1(
                relu(((Src0 * C0 + C1) * Src0 + C2) * Src0 + C3)
            ),
            reference=lambda in0, in1, s0, s1, imm2: np.maximum(
                ((in0 * s0 + s1) * in0 + imm2) * in0 + in1, 0.0
            ).astype(np.float32),
        ),
    )
    p2 = reg(
        "POW16_ANT",
        Spec(
            body=sq(sq(sq(sq(Src0)))),
            reference=lambda in0, in1, s0, s1, imm2: (in0 ** 16).astype(
                np.float32
            ),
        ),
    )
    return p1, p2


try:
    _EXP16_P1, _POW16 = _register_exp_ops()
    OFFLOAD = {4, 7}  # tiles per (nb, h) group whose exp runs on VectorE
except Exception:
    _EXP16_P1 = _POW16 = None
    OFFLOAD = set()


def build_nc():
    nc = bacc.Bacc("TRN2", target_bir_lowering=False)
    xT = nc.declare_dram_parameter("xT", [128, CC * N], BF16, isOutput=False)
    wqkvT = nc.declare_dram_parameter("wqkvT", [128, CC * 384], BF16, isOutput=False)
    wpT = nc.declare_dram_parameter("wpT", [128, DIM], BF16, isOutput=False)
    # out2[:, nb*2048 + ob*512 + q] = y_partial^T[ob*128:, nb*512 + q]
    out = nc.declare_dram_parameter("out", [128, 4 * N], F32, isOutput=True)

    with ExitStack() as ctx:
        tc = ctx.enter_context(tile.TileContext(nc))
        big = ctx.enter_context(tc.tile_pool(name="big", bufs=1))
        esp = ctx.enter_context(tc.tile_pool(name="esp", bufs=5))
        p1p = ctx.enter_context(tc.tile_pool(name="p1p", bufs=2))
        yup = ctx.enter_context(tc.tile_pool(name="yup", bufs=2))
        ysp = ctx.enter_context(tc.tile_pool(name="ysp", bufs=2))
        ps_p = ctx.enter_context(tc.tile_pool(name="psA", bufs=2, space="PSUM"))
        po_p = ctx.enter_context(tc.tile_pool(name="psB", bufs=2, space="PSUM"))

        # ---- packed bf16 loads: one long contiguous DRAM run per partition
        wqt = big.tile([128, CC * 384], BF16, tag="wqt", name="wqt")
        nc.gpsimd.dma_start(out=wqt[:], in_=wqkvT[:, :])
        wq = [wqt[:, cc * 384:(cc + 1) * 384] for cc in range(CC)]
        xtb_t = big.tile([128, CC * N], BF16, tag="xtb", name="xtb")
        nc.sync.dma_start(out=xtb_t[:], in_=xT[:, :])
        xtb = [xtb_t[:, cc * N:(cc + 1) * N] for cc in range(CC)]
        wpb = big.tile([128, DIM], BF16, tag="wpb", name="wpb")
        nc.gpsimd.dma_start(out=wpb[:], in_=wpT[:, :])

        # V2 layout per m-chunk: [V_h0(64) | 1 | V_h1(64) | 1]
        v2 = big.tile([128, 130 * MB], BF16, tag="v2", name="v2")
        nc.vector.memset(v2[:], 1.0)
        qt = big.tile([128, N], BF16, tag="qt", name="qt")
        kt = [big.tile([128, N], BF16, tag=f"kt{h}", name=f"kt{h}") for h in range(2)]
        nc.vector.memset(kt[0][64:128, :], 0.0)
        nc.vector.memset(kt[1][0:64, :], 0.0)
        atB = big.tile([128, N], BF16, tag="atB", name="atB")
        if OFFLOAD:
            c0t = big.tile([128, 1], F32, tag="c0t", name="c0t")
            nc.vector.memset(c0t[:], _EC0)

        # ---- projection prologue ----
        def emit_kq(blk, nb):
            ps = po_p.tile([128, 512], F32, tag="po", name="po")
            for cc in range(CC):
                nc.tensor.matmul(
                    ps[:],
                    lhsT=wq[cc][:, blk * 128:(blk + 1) * 128],
                    rhs=xtb[cc][:, nb * 512:(nb + 1) * 512],
                    start=(cc == 0),
                    stop=(cc == CC - 1),
                )
            ns = slice(nb * 512, (nb + 1) * 512)
            # ScalarE is idle through the whole prologue and reads PSUM
            # natively; keeping these copies off VectorE/GpSimd leaves both
            # free for the attention steady state.
            if blk == 0:
                nc.scalar.copy(out=qt[:, ns], in_=ps[:])
            else:
                nc.scalar.copy(out=kt[0][0:64, ns], in_=ps[0:64, :])
                nc.scalar.copy(out=kt[1][64:128, ns], in_=ps[64:128, :])

        def emit_v4(mb0):
            # four 128-key V chunks into one psum tile
            ps = po_p.tile([128, 512], F32, tag="po", name="po")
            for j in range(4):
                mb = mb0 + j
                for cc in range(CC):
                    nc.tensor.matmul(
                        ps[:, j * 128:j * 128 + 128],
                        lhsT=xtb[cc][:, mb * 128:(mb + 1) * 128],
                        rhs=wq[cc][:, 256:384],
                        start=(cc == 0),
                        stop=(cc == CC - 1),
                    )
            for j in range(4):
                mb = mb0 + j
                nc.scalar.copy(
                    out=v2[:, mb * 130:mb * 130 + 64], in_=ps[:, j * 128:j * 128 + 64]
                )
                nc.scalar.copy(
                    out=v2[:, mb * 130 + 65:mb * 130 + 129],
                    in_=ps[:, j * 128 + 64:j * 128 + 128],
                )

        for nb in range(NB):
            emit_kq(1, nb)
        emit_kq(0, 0)
        for mb0 in range(0, MB, 4):
            emit_v4(mb0)
        for nb in range(1, NB):
            emit_kq(0, nb)

        # ---- attention + partial proj per n-block ----
        def emit_proj(nb):
            ns = slice(nb * 512, (nb + 1) * 512)
            ys = ysp.tile([128, 2048], F32, tag="ys", name="ys")
            for ob in range(4):
                pp = po_p.tile([128, 512], F32, tag="po", name="pp")
                nc.tensor.matmul(
                    pp[:],
                    lhsT=wpb[:, ob * 128:(ob + 1) * 128],
                    rhs=atB[:, ns],
                    start=True,
                    stop=True,
                )
                nc.vector.tensor_copy(out=ys[:, ob * 512:(ob + 1) * 512], in_=pp[:])
            oq = nc.sync if nb % 2 == 0 else nc.gpsimd
            oq.dma_start(out=out[:, nb * 2048:(nb + 1) * 2048], in_=ys[:])

        def emit_norm(nb, h, po):
            ns = slice(nb * 512, (nb + 1) * 512)
            yu = yup.tile([128, 512], F32, tag="yu", name="yu")
            nc.vector.tensor_copy(out=yu[0:65, :], in_=po[0:65, :])
            row = yup.tile([1, 512], F32, tag="row", name="row")
            nc.sync.dma_start(out=row[:], in_=yu[64:65, :])
            den = yup.tile([64, 512], F32, tag="den", name="den")
            nc.gpsimd.partition_broadcast(den[:], row[0:1, :])
            rec = yup.tile([64, 512], F32, tag="rec", name="rec")
            nc.vector.reciprocal_approx_fast(out=rec[:], in_=den[:])
            if h == 0:
                nc.vector.tensor_mul(out=atB[0:64, ns], in0=yu[0:64, :], in1=rec[:])
                if nb > 0:
                    emit_proj(nb - 1)
            else:
                a1 = yup.tile([64, 512], BF16, tag="a1", name="a1")
                nc.vector.tensor_mul(out=a1[:], in0=yu[0:64, :], in1=rec[:])
                nc.sync.dma_start(out=atB[64:128, ns], in_=a1[:])

        def emit_pv(po, h, mb0, w, es):
            for j in range(w):
                mb = mb0 + j
                nc.tensor.matmul(
                    po[0:65, :],
                    lhsT=v2[:, mb * 130 + 65 * h:mb * 130 + 65 * h + 65],
                    rhs=es[:, j * 512:(j + 1) * 512],
                    start=(mb == 0),
                    stop=(mb == MB - 1),
                )

        pend = None  # (nb, h, po, mb0, w, es)
        for nb in range(NB):
            ns = slice(nb * 512, (nb + 1) * 512)
            for h in range(2):
                po = po_p.tile([128, 512], F32, tag="po", name="po")
                pvq = []  # (due_tile, mb0, w, es)
                for t, (mb0, w) in enumerate(TILES):
                    ps = ps_p.tile([128, 1536], F32, tag="ps", name="ps")
                    for j in range(w):
                        mb = mb0 + j
                        nc.tensor.matmul(
                            ps[:, j * 512:(j + 1) * 512],
                            lhsT=kt[h][:, mb * 128:(mb + 1) * 128],
                            rhs=qt[:, ns],
                            start=True,
                            stop=True,
                        )
                    es = esp.tile([128, 1536], BF16, tag="es", name="es")
                    if t in OFFLOAD:
                        p1 = p1p.tile([128, 1536], F32, tag="p1", name="p1")
                        nc.vector._custom_dve(
                            _EXP16_P1,
                            out=p1[:, 0:w * 512],
                            in0=ps[:, 0:w * 512],
                            s0=_EC3,
                            s1=_EC2,
                            imm2=_EC1,
                            in1=c0t[:],
                        )
                        nc.vector._custom_dve(
                            _POW16, out=es[:, 0:w * 512], in0=p1[:, 0:w * 512]
                        )
                        due = t + 3
                    else:
                        nc.scalar.activation(
                            out=es[:, 0:w * 512],
                            in_=ps[:, 0:w * 512],
                            func=EXP,
                            scale=SCALE,
                        )
                        due = t + 1
                    if t == 0 and pend is not None:
                        # flush the previous group: its last PV + norm run
                        # while this group's first exp is still in flight
                        pnb, ph, ppo, pmb0, pw, pes = pend
                        emit_pv(ppo, ph, pmb0, pw, pes)
                        emit_norm(pnb, ph, ppo)
                        pend = None
                    while pvq and pvq[0][0] <= t:
                        _, dmb0, dw, des = pvq.pop(0)
                        emit_pv(po, h, dmb0, dw, des)
                    pvq.append((due, mb0, w, es))
                # drain all but the last tile's PV; that one flushes in the
                # next group so the PE isn't stalled on this group's last exp
                while len(pvq) > 1:
                    _, dmb0, dw, des = pvq.pop(0)
                    emit_pv(po, h, dmb0, dw, des)
                _, lmb0, lw, les = pvq.pop()
                pend = (nb, h, po, lmb0, lw, les)
        pnb, ph, ppo, pmb0, pw, pes = pend
        emit_pv(ppo, ph, pmb0, pw, pes)
        emit_norm(pnb, ph, ppo)
        emit_proj(NB - 1)

    nc.compile()
    return nc


_NC_CACHE = None
LAST_EXEC_NS = None


def kernel(x, w_qkv, w_proj, b_proj):
    global _NC_CACHE, LAST_EXEC_NS
    x = np.ascontiguousarray(np.asarray(x, dtype=np.float32))
    w_qkv = np.asarray(w_qkv, dtype=np.float32)
    w_proj = np.asarray(w_proj, dtype=np.float32)
    b_proj = np.asarray(b_proj, dtype=np.float32)
    B = x.shape[0]

    if _NC_CACHE is None:
        _NC_CACHE = build_nc()
    nc = _NC_CACHE

    bf16 = ml_dtypes.bfloat16
    # pack x^T per batch: [128, cc*4096] with cc blocks side by side
    xTs = []
    for b in range(B):
        xt = x[b].T.astype(bf16)  # [512, 4096]
        xTs.append(
            np.ascontiguousarray(
                np.concatenate([xt[cc * 128:(cc + 1) * 128, :] for cc in range(CC)], 1)
            )
        )
    in_maps = []
    for c in range(8):
        b, hp = c // 4, c % 4
        qr = w_qkv[2 * hp * 64:2 * hp * 64 + 128]
        kr = w_qkv[512 + 2 * hp * 64:512 + 2 * hp * 64 + 128]
        vr = w_qkv[1024 + 2 * hp * 64:1024 + 2 * hp * 64 + 128]
        wqkvT_full = np.concatenate([qr, kr, vr], 0).T.astype(bf16)  # [512, 384]
        wqkvT = np.ascontiguousarray(
            np.concatenate(
                [wqkvT_full[cc * 128:(cc + 1) * 128, :] for cc in range(CC)], 1
            )
        )
        wpT = np.ascontiguousarray(w_proj[:, hp * 128:(hp + 1) * 128].T.astype(bf16))
        in_maps.append({"xT": xTs[b], "wqkvT": wqkvT, "wpT": wpT})

    res = run_bass_kernel_spmd(
        nc,
        in_maps,
        core_ids=list(range(8)),
        trace=bool(int(os.environ.get("ATTN_TRACE", "0"))),
    )
    LAST_EXEC_NS = res.exec_time_ns

    out = np.zeros((B, N, DIM), np.float32)
    for b in range(B):
        acc = res.results[4 * b]["out"].copy()
        for c in range(4 * b + 1, 4 * b + 4):
            acc += res.results[c]["out"]
        # unpack [128, nb*2048 + ob*512 + q] -> y^T [512, 4096] -> y
        yT = np.empty((DIM, N), np.float32)
        for nb in range(NB):
            for ob in range(4):
                yT[ob * 128:(ob + 1) * 128, nb * 512:(nb + 1) * 512] = acc[
                    :, nb * 2048 + ob * 512: nb * 2048 + (ob + 1) * 512
                ]
        out[b] = yT.T + b_proj
    return out


# revision 19
# speedup vs baseline: 1.0368x; 1.0368x over previous
"""Multi-head attention (B=2, N=4096, C=512, H=8, D=64) on 8 TRN2 NeuronCores.

Sharding: data-parallel over batch (2 groups of 4 cores) x tensor-parallel over
heads (2 heads/core). Per core: qkv projection, attention for its 2 heads, and
a partial output projection y_partial^T = Wp_slice^T @ attn^T; the host sums
the 4 per-batch partials, transposes, adds bias.

Engine layout learned from profiling:
- All matmuls bf16; x / w_qkv / w_proj are cast to bf16 on the HOST.
- DMA descriptor dispatch (~30-40 ns/descriptor, shared DGE) paces all loads
  and stores, so every DRAM tensor is packed so each SBUF partition's data is
  one long contiguous DRAM run: x is a single [128, 16384] transfer
  (128 x 32 KB descriptors), wqkv one [128, 1536], and the output is stored
  as [128, 2048] rows per n-block (8 KB descriptors).
- exp(S) runs on ScalarE out of 3-bank (128,1536) PSUM tiles at the
  documented (cols+352)/1.2 ns; two tiles per group are offloaded to VectorE
  via a custom 2-instruction exp: p = relu(cubic(s)) then p^16 (4 squarings),
  which matches exp(SCALE*s) to ~4e-4 over the live logit range. Their PV
  matmuls are deferred 3 tiles to cover the longer DVE latency.
- Softmax denominator rides the PV matmul as a ones-column appended to V
  (lhsT is [V_h | 1], M=65); the denominator row is moved to partition 0 by a
  small DMA, inverted on VectorE, and partition-broadcast on GpSimd.
"""
import os
import sys

for _p in ("/opt/trn_rl_repo", "/root/.axon_site/_ro/trn_rl_repo"):
    if os.path.isdir(_p) and _p not in sys.path:
        sys.path.append(_p)

import numpy as np
import ml_dtypes
from contextlib import ExitStack

import concourse.bass as bass
import concourse.mybir as mybir
import concourse.tile as tile
from concourse import bacc
from concourse.bass_utils import run_bass_kernel_spmd

F32 = mybir.dt.float32
BF16 = mybir.dt.bfloat16
EXP = mybir.ActivationFunctionType.Exp

DIM, N, HD = 512, 4096, 64
SCALE = HD ** -0.5
NB = N // 512    # 8  n-blocks of 512 queries
MB = N // 128    # 32 m-chunks of 128 keys
CC = DIM // 128  # 4  c-chunks of the model dim
# m-chunks per (nb, h) are processed in 3-bank PSUM tiles of 3 chunks each
TILES = [(t * 3, 3) for t in range(10)] + [(30, 2)]

# ---- custom DVE exp: es = (relu(cubic(s)))^16 ~= exp(SCALE*s) -------------
# cubic fit of 2^(k*s), k = SCALE*log2(e)/16, minimax-relative over logits
# s*SCALE in [-9.5, 7.2]; worst rel err ^16 ~ 6e-3, and it sits at large
# negative logits where softmax weights are ~0 (full-pipeline err is ~3e-3).
_EC3, _EC2, _EC1, _EC0 = (
    7.29580735e-08, 3.10468742e-05, 7.82950121e-03, 9.99782936e-01,
)


def _register_exp_ops():
    from concourse import dve_ops as dvo
    from concourse.dve_spec import Spec, Src0, relu, sq, C0, C1, C2, C3

    def reg(name, spec, subdim=False):
        for o in dvo.OPS:
            if o.name == name:
                return o
        opcode = max(dvo._SUB_OPCODE_FOR_NAME.values()) + 1
        from concourse.dve_uop import DveOpSpec

        shas = {}
        for ver in ("v3", "v4"):
            try:
                r = DveOpSpec(
                    name=name,
                    opcode=opcode,
                    uops=dvo.lower(spec, ver=ver),
                    rd1_en=dvo.has_src1(spec),
                )
                shas[ver] = r.sha(ver)
            except Exception:
                pass
        op = dvo.DveOp(name, spec, subdim=subdim, uops_sha=shas)
        dvo.OPS.append(op)
        dvo.CUSTOM_DVE_SPECS[name] = spec
        dvo._SUB_OPCODE_FOR_NAME[name] = opcode
        return op

    p1 = reg(
        "EXP16_P1_ANT",
        Spec(
            body=dvo._spill_c3_to_src1(
                relu(((Src0 * C0 + C1) * Src0 + C2) * Src0 + C3)
            ),
            reference=lambda in0, in1, s0, s1, imm2: np.maximum(
                ((in0 * s0 + s1) * in0 + imm2) * in0 + in1, 0.0
            ).astype(np.float32),
        ),
    )
    p2 = reg(
        "POW16_ANT",
        Spec(
            body=sq(sq(sq(sq(Src0)))),
            reference=lambda in0, in1, s0, s1, imm2: (in0 ** 16).astype(
                np.float32
            ),
        ),
    )
    return p1, p2


try:
    _EXP16_P1, _POW16 = _register_exp_ops()
    # VectorE exp offload measured slower end-to-end (FIFO collisions with
    # the norm/copy chains starve the psA ring); keep exp on ScalarE.
    OFFLOAD = set()
except Exception:
    _EXP16_P1 = _POW16 = None
    OFFLOAD = set()


def build_nc():
    nc = bacc.Bacc("TRN2", target_bir_lowering=False)
    xT = nc.declare_dram_parameter("xT", [128, CC * N], BF16, isOutput=False)
    wqkvT = nc.declare_dram_parameter("wqkvT", [128, CC * 384], BF16, isOutput=False)
    wpT = nc.declare_dram_parameter("wpT", [128, DIM], BF16, isOutput=False)
    # out2[:, nb*2048 + ob*512 + q] = y_partial^T[ob*128:, nb*512 + q]
    out = nc.declare_dram_parameter("out", [128, 4 * N], F32, isOutput=True)

    with ExitStack() as ctx:
        tc = ctx.enter_context(tile.TileContext(nc))
        big = ctx.enter_context(tc.tile_pool(name="big", bufs=1))
        esp = ctx.enter_context(tc.tile_pool(name="esp", bufs=5))
        p1p = ctx.enter_context(tc.tile_pool(name="p1p", bufs=2))
        yup = ctx.enter_context(tc.tile_pool(name="yup", bufs=2))
        ysp = ctx.enter_context(tc.tile_pool(name="ysp", bufs=2))
        ps_p = ctx.enter_context(tc.tile_pool(name="psA", bufs=2, space="PSUM"))
        po_p = ctx.enter_context(tc.tile_pool(name="psB", bufs=2, space="PSUM"))

        # ---- packed bf16 loads: one long contiguous DRAM run per partition
        wqt = big.tile([128, CC * 384], BF16, tag="wqt", name="wqt")
        nc.gpsimd.dma_start(out=wqt[:], in_=wqkvT[:, :])
        wq = [wqt[:, cc * 384:(cc + 1) * 384] for cc in range(CC)]
        xtb_t = big.tile([128, CC * N], BF16, tag="xtb", name="xtb")
        nc.sync.dma_start(out=xtb_t[:], in_=xT[:, :])
        xtb = [xtb_t[:, cc * N:(cc + 1) * N] for cc in range(CC)]
        wpb = big.tile([128, DIM], BF16, tag="wpb", name="wpb")
        nc.gpsimd.dma_start(out=wpb[:], in_=wpT[:, :])

        # V2 layout per m-chunk: [V_h0(64) | 1 | V_h1(64) | 1]
        v2 = big.tile([128, 130 * MB], BF16, tag="v2", name="v2")
        nc.vector.memset(v2[:], 1.0)
        qt = big.tile([128, N], BF16, tag="qt", name="qt")
        kt = [big.tile([128, N], BF16, tag=f"kt{h}", name=f"kt{h}") for h in range(2)]
        nc.vector.memset(kt[0][64:128, :], 0.0)
        nc.vector.memset(kt[1][0:64, :], 0.0)
        atB = big.tile([128, N], BF16, tag="atB", name="atB")
        if OFFLOAD:
            c0t = big.tile([128, 1], F32, tag="c0t", name="c0t")
            nc.vector.memset(c0t[:], _EC0)

        # ---- projection prologue ----
        def emit_kq(blk, nb):
            ps = po_p.tile([128, 512], F32, tag="po", name="po")
            for cc in range(CC):
                nc.tensor.matmul(
                    ps[:],
                    lhsT=wq[cc][:, blk * 128:(blk + 1) * 128],
                    rhs=xtb[cc][:, nb * 512:(nb + 1) * 512],
                    start=(cc == 0),
                    stop=(cc == CC - 1),
                )
            ns = slice(nb * 512, (nb + 1) * 512)
            if blk == 0:
                nc.vector.tensor_copy(out=qt[:, ns], in_=ps[:])
            else:
                nc.vector.tensor_copy(out=kt[0][0:64, ns], in_=ps[0:64, :])
                nc.vector.tensor_copy(out=kt[1][64:128, ns], in_=ps[64:128, :])

        def emit_v4(mb0):
            # four 128-key V chunks into one psum tile
            ps = po_p.tile([128, 512], F32, tag="po", name="po")
            for j in range(4):
                mb = mb0 + j
                for cc in range(CC):
                    nc.tensor.matmul(
                        ps[:, j * 128:j * 128 + 128],
                        lhsT=xtb[cc][:, mb * 128:(mb + 1) * 128],
                        rhs=wq[cc][:, 256:384],
                        start=(cc == 0),
                        stop=(cc == CC - 1),
                    )
            for j in range(4):
                mb = mb0 + j
                nc.vector.tensor_copy(
                    out=v2[:, mb * 130:mb * 130 + 64], in_=ps[:, j * 128:j * 128 + 64]
                )
                nc.vector.tensor_copy(
                    out=v2[:, mb * 130 + 65:mb * 130 + 129],
                    in_=ps[:, j * 128 + 64:j * 128 + 128],
                )

        for nb in range(NB):
            emit_kq(1, nb)
        emit_kq(0, 0)
        for mb0 in range(0, MB, 4):
            emit_v4(mb0)
        for nb in range(1, NB):
            emit_kq(0, nb)

        # ---- attention + partial proj per n-block ----
        def emit_proj(nb):
            ns = slice(nb * 512, (nb + 1) * 512)
            ys = ysp.tile([128, 2048], F32, tag="ys", name="ys")
            for ob in range(4):
                pp = po_p.tile([128, 512], F32, tag="po", name="pp")
                nc.tensor.matmul(
                    pp[:],
                    lhsT=wpb[:, ob * 128:(ob + 1) * 128],
                    rhs=atB[:, ns],
                    start=True,
                    stop=True,
                )
                nc.vector.tensor_copy(out=ys[:, ob * 512:(ob + 1) * 512], in_=pp[:])
            oq = nc.sync if nb % 2 == 0 else nc.gpsimd
            oq.dma_start(out=out[:, nb * 2048:(nb + 1) * 2048], in_=ys[:])

        def emit_norm(nb, h, po):
            ns = slice(nb * 512, (nb + 1) * 512)
            yu = yup.tile([128, 512], F32, tag="yu", name="yu")
            nc.vector.tensor_copy(out=yu[0:65, :], in_=po[0:65, :])
            row = yup.tile([1, 512], F32, tag="row", name="row")
            nc.sync.dma_start(out=row[:], in_=yu[64:65, :])
            den = yup.tile([64, 512], F32, tag="den", name="den")
            nc.gpsimd.partition_broadcast(den[:], row[0:1, :])
            rec = yup.tile([64, 512], F32, tag="rec", name="rec")
            nc.vector.reciprocal_approx_fast(out=rec[:], in_=den[:])
            if h == 0:
                nc.vector.tensor_mul(out=atB[0:64, ns], in0=yu[0:64, :], in1=rec[:])
                if nb > 0:
                    emit_proj(nb - 1)
            else:
                a1 = yup.tile([64, 512], BF16, tag="a1", name="a1")
                nc.vector.tensor_mul(out=a1[:], in0=yu[0:64, :], in1=rec[:])
                nc.sync.dma_start(out=atB[64:128, ns], in_=a1[:])

        def emit_pv(po, h, mb0, w, es):
            for j in range(w):
                mb = mb0 + j
                nc.tensor.matmul(
                    po[0:65, :],
                    lhsT=v2[:, mb * 130 + 65 * h:mb * 130 + 65 * h + 65],
                    rhs=es[:, j * 512:(j + 1) * 512],
                    start=(mb == 0),
                    stop=(mb == MB - 1),
                )

        pend = None  # (nb, h, po, mb0, w, es)
        for nb in range(NB):
            ns = slice(nb * 512, (nb + 1) * 512)
            for h in range(2):
                po = po_p.tile([128, 512], F32, tag="po", name="po")
                pvq = []  # (due_tile, mb0, w, es)
                for t, (mb0, w) in enumerate(TILES):
                    ps = ps_p.tile([128, 1536], F32, tag="ps", name="ps")
                    for j in range(w):
                        mb = mb0 + j
                        nc.tensor.matmul(
                            ps[:, j * 512:(j + 1) * 512],
                            lhsT=kt[h][:, mb * 128:(mb + 1) * 128],
                            rhs=qt[:, ns],
                            start=True,
                            stop=True,
                        )
                    es = esp.tile([128, 1536], BF16, tag="es", name="es")
                    if t in OFFLOAD:
                        p1 = p1p.tile([128, 1536], F32, tag="p1", name="p1")
                        nc.vector._custom_dve(
                            _EXP16_P1,
                            out=p1[:, 0:w * 512],
                            in0=ps[:, 0:w * 512],
                            s0=_EC3,
                            s1=_EC2,
                            imm2=_EC1,
                            in1=c0t[:],
                        )
                        nc.vector._custom_dve(
                            _POW16, out=es[:, 0:w * 512], in0=p1[:, 0:w * 512]
                        )
                        due = t + 3
                    else:
                        nc.scalar.activation(
                            out=es[:, 0:w * 512],
                            in_=ps[:, 0:w * 512],
                            func=EXP,
                            scale=SCALE,
                        )
                        due = t + 1
                    if t == 0 and pend is not None:
                        # flush the previous group: its last PV + norm run
                        # while this group's first exp is still in flight
                        pnb, ph, ppo, pmb0, pw, pes = pend
                        emit_pv(ppo, ph, pmb0, pw, pes)
                        emit_norm(pnb, ph, ppo)
                        pend = None
                    while pvq and pvq[0][0] <= t:
                        _, dmb0, dw, des = pvq.pop(0)
                        emit_pv(po, h, dmb0, dw, des)
                    pvq.append((due, mb0, w, es))
                # drain all but the last tile's PV; that one flushes in the
                # next group so the PE isn't stalled on this group's last exp
                while len(pvq) > 1:
                    _, dmb0, dw, des = pvq.pop(0)
                    emit_pv(po, h, dmb0, dw, des)
                _, lmb0, lw, les = pvq.pop()
                pend = (nb, h, po, lmb0, lw, les)
        pnb, ph, ppo, pmb0, pw, pes = pend
        emit_pv(ppo, ph, pmb0, pw, pes)
        emit_norm(pnb, ph, ppo)
        emit_proj(NB - 1)

    nc.compile()
    return nc


_NC_CACHE = None
LAST_EXEC_NS = None


def kernel(x, w_qkv, w_proj, b_proj):
    global _NC_CACHE, LAST_EXEC_NS
    x = np.ascontiguousarray(np.asarray(x, dtype=np.float32))
    w_qkv = np.asarray(w_qkv, dtype=np.float32)
    w_proj = np.asarray(w_proj, dtype=np.float32)
    b_proj = np.asarray(b_proj, dtype=np.float32)
    B = x.shape[0]

    if _NC_CACHE is None:
        _NC_CACHE = build_nc()
    nc = _NC_CACHE

    bf16 = ml_dtypes.bfloat16
    # pack x^T per batch: [128, cc*4096] with cc blocks side by side
    xTs = []
    for b in range(B):
        xt = x[b].T.astype(bf16)  # [512, 4096]
        xTs.append(
            np.ascontiguousarray(
                np.concatenate([xt[cc * 128:(cc + 1) * 128, :] for cc in range(CC)], 1)
            )
        )
    in_maps = []
    for c in range(8):
        b, hp = c // 4, c % 4
        qr = w_qkv[2 * hp * 64:2 * hp * 64 + 128]
        kr = w_qkv[512 + 2 * hp * 64:512 + 2 * hp * 64 + 128]
        vr = w_qkv[1024 + 2 * hp * 64:1024 + 2 * hp * 64 + 128]
        wqkvT_full = np.concatenate([qr, kr, vr], 0).T.astype(bf16)  # [512, 384]
        wqkvT = np.ascontiguousarray(
            np.concatenate(
                [wqkvT_full[cc * 128:(cc + 1) * 128, :] for cc in range(CC)], 1
            )
        )
        wpT = np.ascontiguousarray(w_proj[:, hp * 128:(hp + 1) * 128].T.astype(bf16))
        in_maps.append({"xT": xTs[b], "wqkvT": wqkvT, "wpT": wpT})

    res = run_bass_kernel_spmd(
        nc,
        in_maps,
        core_ids=list(range(8)),
        trace=bool(int(os.environ.get("ATTN_TRACE", "0"))),
    )
    LAST_EXEC_NS = res.exec_time_ns

    out = np.zeros((B, N, DIM), np.float32)
    for b in range(B):
        acc = res.results[4 * b]["out"].copy()
        for c in range(4 * b + 1, 4 * b + 4):
            acc += res.results[c]["out"]
        # unpack [128, nb*2048 + ob*512 + q] -> y^T [512, 4096] -> y
        yT = np.empty((DIM, N), np.float32)
        for nb in range(NB):
            for ob in range(4):
                yT[ob * 128:(ob + 1) * 128, nb * 512:(nb + 1) * 512] = acc[
                    :, nb * 2048 + ob * 512: nb * 2048 + (ob + 1) * 512
                ]
        out[b] = yT.T + b_proj
    return out
